# revision 13
# baseline (speedup 1.0000x reference)
"""MeshMeanFlowNet block on 8 Trainium2 NeuronCores.

Sharding: data-parallel over B (one batch element per core), no collectives.
All activations are kept feature-major on device ([feature, token]) so every
linear layer consumes its input directly as the matmul moving operand and
produces feature-major output. The attention softmax is computed in the
transposed layout S^T[j, i] (j = key token on partitions, i = query token on
the free axis); the softmax denominator comes for free from a ones-row
appended to V, so no on-device transposes are needed anywhere.

The per-edge-type/per-head bias is accumulated INTO the score PSUM by the
tensor engine: the host sends one-hot edge masks packed as fp8 pairs, and per
(key-tile, head) the kernel issues fp8 DoubleRow matmuls
   S += (c0h*I | c1h*I)^T @ (m0 | m1)  and  S += (c2h*I | 0)^T @ (m2 | 0)
so exp() reads fully-biased logits straight from PSUM.

The qkv / v / proj projections run as fp8 DoubleRow matmuls (weights
pre-scaled by 32 on the host; the inverse scale is folded into the fused
PSUM-consuming op downstream). The MLP stays f32r: fp8 there would cost
~1e-2 relative error against the 2e-2 gate.
"""

import sys

sys.path.insert(0, "/opt/trn_rl_repo")

import ml_dtypes
import numpy as np

B, V, D, H = 8, 1024, 512, 8
HD = D // H  # 64
NCORES = 8
WS = 32.0      # fp8 weight pre-scale
AS = 8.0       # fp8 att pre-scale
QS = 16.0      # fp8 q/k pre-scale (S psum carries QS^2; exp rescales)

_cache = {}


def _build_program():
    import contextlib

    import concourse.bacc as bacc
    import concourse.tile as tile
    from concourse import mybir

    f32 = mybir.dt.float32
    f32r = mybir.dt.float32r
    bf16 = mybir.dt.bfloat16
    f8 = mybir.dt.float8e4
    f8e5 = mybir.dt.float8e5
    ALU = mybir.AluOpType
    ACTF = mybir.ActivationFunctionType
    DR = mybir.MatmulPerfMode.DoubleRow

    nc = bacc.Bacc("TRN2", target_bir_lowering=False, debug=False,
                   num_devices=NCORES)

    # ---- DRAM I/O (per-core shard, host pre-laid-out) ----
    xT = nc.dram_tensor("xT", [D, V], f32r, kind="ExternalInput")
    # packed one-hot masks per key j: [m0 | m1 | m2 | 0], each V wide
    mk = nc.dram_tensor("mk", [V, 4 * V], f8, kind="ExternalInput")
    # per-head identity blocks: per h (512 cols): [c0h*I | c1h*I | c2h*I | 0]
    wid = nc.dram_tensor("wid", [128, 4096], f8, kind="ExternalInput")
    condc = nc.dram_tensor("condc", [4, 128], f32, kind="ExternalInput")
    # fp8 DoubleRow pair layouts: row = kp*128 + k, col = t*M + m
    wqk8 = nc.dram_tensor("wqk8", [256, 2048], f8, kind="ExternalInput")
    wv8 = nc.dram_tensor("wv8", [256, 1024], f8, kind="ExternalInput")
    wp8 = nc.dram_tensor("wp8", [256, 1024], f8, kind="ExternalInput")
    wada = nc.dram_tensor("wada", [D, 2048], bf16, kind="ExternalInput")
    bada = nc.dram_tensor("bada", [16, 128], f32, kind="ExternalInput")
    wm1 = nc.dram_tensor("wm1", [D, 2048], f32r, kind="ExternalInput")
    bm1 = nc.dram_tensor("bm1", [16, 128], f32, kind="ExternalInput")
    wm2 = nc.dram_tensor("wm2", [2048, D], f32r, kind="ExternalInput")
    bm2 = nc.dram_tensor("bm2", [4, 128], f32, kind="ExternalInput")
    onesc = nc.dram_tensor("onesc", [128, 8], f32r, kind="ExternalInput")
    ones16 = nc.dram_tensor("ones16", [128, 16], f8, kind="ExternalInput")
    yT = nc.dram_tensor("yT", [D, V], f32, kind="ExternalOutput")

    def mm(out, lhsT, rhs, **kw):
        nc.tensor.matmul(out, lhsT.bitcast(f32r), rhs.bitcast(f32r), **kw)

    with tile.TileContext(nc) as tc:
        with contextlib.ExitStack() as ctx:
            persist = ctx.enter_context(tc.tile_pool(name="persist", bufs=1))

            ones = persist.tile([128, 1], f32r, tag="ones")
            nc.sync.dma_start(out=ones, in_=onesc[:, 0:1])
            onesrow = persist.tile([1, 128], f32r, tag="onesrow")
            nc.sync.dma_start(out=onesrow,
                              in_=onesc[:, 0:1].rearrange("p o -> o p"))
            epst = persist.tile([1, 1], f32, tag="eps")
            nc.vector.memset(epst, 1e-5)

            # x (feature-major)
            xT_t = [persist.tile([128, V], f32r, tag=f"xT{kc}",
                                 name=f"xT_t{kc}") for kc in range(4)]
            for kc in range(4):
                nc.sync.dma_start(out=xT_t[kc],
                                  in_=xT[kc * 128:(kc + 1) * 128, :])

            bada_t = persist.tile([128, 16], f32, tag="bada")
            nc.sync.dma_start(out=bada_t, in_=bada[:].rearrange("c p -> p c"))
            bm1_t = persist.tile([128, 16], f32, tag="bm1")
            nc.sync.dma_start(out=bm1_t, in_=bm1[:].rearrange("c p -> p c"))
            bm2_t = persist.tile([128, 4], f32, tag="bm2")
            nc.sync.dma_start(out=bm2_t, in_=bm2[:].rearrange("c p -> p c"))

            x2 = [persist.tile([128, V], f32r, tag=f"x2_{kc}",
                                name=f"x2_{kc}") for kc in range(4)]
            params = persist.tile([128, 16], f32, tag="params")

            # ---------- AdaLN parameter path ----------
            with tc.tile_pool(name="adaw", bufs=1) as adaw, \
                    tc.tile_pool(name="adap", bufs=2, space="PSUM") as adap:
                condt = adaw.tile([128, 5], f32, tag="cond")
                nc.sync.dma_start(out=condt[:, 0:4],
                                  in_=condc[:].rearrange("c p -> p c"))
                nc.vector.memset(condt[:, 4:5], 0.0)
                scond = adaw.tile([128, 5], bf16, tag="scond")
                nc.scalar.activation(scond, condt, ACTF.Silu)
                wada_t = [adaw.tile([128, 2048], bf16, tag=f"wada{kc}",
                                    name="wada_t") for kc in range(4)]
                for kc in range(4):
                    nc.sync.dma_start(out=wada_t[kc],
                                      in_=wada[kc * 128:(kc + 1) * 128, :])
                pp = adap.tile([2, 2048], f32, tag="pada")
                for oc in range(4):
                    s = slice(oc * 512, oc * 512 + 512)
                    for kc in range(4):
                        nc.tensor.matmul(pp[:, s], scond[:, kc:kc + 2],
                                         wada_t[kc][:, s],
                                         start=(kc == 0), stop=(kc == 3))
                prow = adaw.tile([1, 2048], f32, tag="prow")
                nc.scalar.copy(prow, pp[0:1, :])
                pscat = adaw.tile([128, 16], f32, tag="pscat")
                for md in range(16):
                    nc.sync.dma_start(
                        out=pscat[:, md:md + 1],
                        in_=prow[0:1, md * 128:(md + 1) * 128])
                nc.vector.tensor_add(params, pscat, bada_t)

            def adaln(src_tiles, ln_idx, out_aps):
                """LayerNorm over the partition (feature) axis + adaptive
                affine from `params`, written into the given 4 dst APs."""
                with tc.tile_pool(name="lnt", bufs=1) as lnt, \
                        tc.tile_pool(name="lnp", bufs=1,
                                     space="PSUM") as lnp:
                    ps_s = lnp.tile([1, V], f32, tag="lnsum")
                    ps_q = lnp.tile([1, V], f32, tag="lnsqsum")
                    for kc in range(4):
                        sq = lnt.tile([128, V], f32r, tag="lnsq", bufs=2,
                                      name="sq")
                        if kc % 2 == 0:
                            nc.scalar.square(sq, src_tiles[kc].bitcast(f32))
                        else:
                            nc.vector.tensor_mul(
                                sq, src_tiles[kc].bitcast(f32),
                                src_tiles[kc].bitcast(f32))
                        for nh in range(2):
                            s = slice(nh * 512, nh * 512 + 512)
                            mm(ps_s[:, s], ones, src_tiles[kc][:, s],
                               start=(kc == 0), stop=(kc == 3))
                            mm(ps_q[:, s], ones, sq[:, s],
                               start=(kc == 0), stop=(kc == 3))
                    mean = lnt.tile([1, V], f32r, tag="mean")
                    nc.scalar.mul(mean, ps_s, 1.0 / D)
                    msq = lnt.tile([1, V], f32, tag="msq")
                    nc.vector.tensor_mul(msq, mean.bitcast(f32),
                                         mean.bitcast(f32))
                    std = lnt.tile([1, V], f32r, tag="std")
                    nc.vector.scalar_tensor_tensor(std, ps_q, 1.0 / D, msq,
                                                   ALU.mult, ALU.subtract)
                    nc.scalar.activation(std, std.bitcast(f32), ACTF.Sqrt,
                                         bias=epst)
                    # broadcast std/mean down the partitions on the PE
                    # (ones-column outer product), then invert on the DVE:
                    # kills the scatter/gather DMA round-trip and the slow
                    # gpsimd broadcasts that serialized this phase.
                    stdb = lnp.tile([128, V], f32, tag="stdb")
                    meanb = lnp.tile([128, V], f32, tag="meanb")
                    for nh in range(2):
                        s = slice(nh * 512, nh * 512 + 512)
                        mm(stdb[:, s], onesrow, std[:, s],
                           start=True, stop=True)
                        mm(meanb[:, s], onesrow, mean[:, s],
                           start=True, stop=True)
                    rb = lnt.tile([128, V], f32, tag="rb")
                    nc.vector.reciprocal(rb, stdb)
                    for kc in range(4):
                        smd = ln_idx * 8 + kc
                        tmd = ln_idx * 8 + 4 + kc
                        u = lnt.tile([128, V], f32, tag="lnu", bufs=2,
                                     name="u")
                        nc.vector.tensor_sub(u, src_tiles[kc].bitcast(f32),
                                             meanb)
                        u2 = lnt.tile([128, V], f32, tag="lnu2", bufs=2,
                                      name="u2")
                        nc.vector.tensor_mul(u2, u, rb)
                        nc.vector.tensor_scalar(out_aps[kc], u2,
                                                params[:, smd:smd + 1],
                                                params[:, tmd:tmd + 1],
                                                ALU.mult, ALU.add)

            # ---- attention-lifetime pool ----
            with tc.tile_pool(name="attlife", bufs=1) as attlife:
                # fp8 q/k in k-split DoubleRow layout [32, 2(dim-half), V]
                q8 = [attlife.tile([32, 2, V], f8, tag=f"q8_{h}",
                                   name=f"q8_{h}") for h in range(8)]
                k8 = [attlife.tile([32, 2, V], f8, tag=f"k8_{h}",
                                   name=f"k8_{h}") for h in range(8)]
                # fp8 staging for the qkv PSUM->fp8 downconvert
                qf8 = [attlife.tile([128, V], f8, tag=f"qf8_{m}",
                                    name=f"qf8_{m}") for m in range(8)]
                # fp8 V in jt-paired DoubleRow layout: per jp, per h:
                # [slot0 64 dims | one] [slot1 64 dims | one]
                vg8 = [attlife.tile([128, 8, 2, 128], f8, tag=f"vg8_{jp}",
                                    name=f"vg8_{jp}") for jp in range(4)]
                # att in fp8 pair layout (scaled by AS)
                att8 = [attlife.tile([128, 2048], f8, tag=f"att8_{kp}",
                                     name=f"att8_{kp}") for kp in range(2)]
                # packed masks + per-head identity blocks (fp8)
                mk_t = [attlife.tile([128, 4 * V], f8, tag=f"mk{jt}",
                                     name=f"mk{jt}") for jt in range(8)]
                for jt in range(8):
                    nc.sync.dma_start(out=mk_t[jt],
                                      in_=mk[jt * 128:(jt + 1) * 128, :])
                wid_t = attlife.tile([128, 4096], f8, tag="wid")
                nc.sync.dma_start(out=wid_t, in_=wid[:])

                # h1 = AdaLN1(x) in fp8 pair layout; qk feature-major;
                # v token-major
                with tc.tile_pool(name="h1pool", bufs=1) as h1pool:
                    h1p = [h1pool.tile([128, 2048], f8, tag=f"h1p{kp}",
                                       name=f"h1p{kp}") for kp in range(2)]
                    adaln(xT_t, 0,
                          [h1p[kc // 2][:, (kc % 2) * 1024:
                                        (kc % 2) * 1024 + 1024]
                           for kc in range(4)])
                    h1r = [h1p[kp][:].rearrange("p (two n) -> p two n",
                                                two=2) for kp in range(2)]
                    with tc.tile_pool(name="qkvw", bufs=1) as qkvw, \
                            tc.tile_pool(name="qkvp", bufs=4,
                                         space="PSUM") as qkvp:
                        wqk_t = [qkvw.tile([128, 2048], f8, tag=f"wqk{kp}",
                                           name="wqk_t") for kp in range(2)]
                        wv_t = [qkvw.tile([128, 1024], f8, tag=f"wv{kp}",
                                          name="wv_t") for kp in range(2)]
                        for kp in range(2):
                            nc.sync.dma_start(
                                out=wqk_t[kp],
                                in_=wqk8[kp * 128:(kp + 1) * 128, :])
                            nc.sync.dma_start(
                                out=wv_t[kp],
                                in_=wv8[kp * 128:(kp + 1) * 128, :])
                        wqk_r = [wqk_t[kp][:].rearrange(
                            "p (two m) -> p two m", two=2)
                            for kp in range(2)]
                        wv_r = [wv_t[kp][:].rearrange(
                            "p (two m) -> p two m", two=2)
                            for kp in range(2)]
                        for m in range(8):
                            for nh in range(2):
                                s = slice(nh * 512, nh * 512 + 512)
                                pp = qkvp.tile([128, 512], f32, tag="mmqk")
                                for kp in range(2):
                                    nc.tensor.matmul(
                                        pp,
                                        wqk_r[kp][:, :,
                                                  m * 128:(m + 1) * 128],
                                        h1r[kp][:, :, s],
                                        start=(kp == 0), stop=(kp == 1),
                                        perf_mode=DR)
                                nc.vector.tensor_scalar(
                                    qf8[m][:, s], pp, QS / WS, None,
                                    ALU.mult)
                        for t in range(8):
                            pp = qkvp.tile([128, 512], f32, tag="mmv")
                            for kp in range(2):
                                nc.tensor.matmul(
                                    pp,
                                    h1r[kp][:, :, t * 128:(t + 1) * 128],
                                    wv_r[kp], start=(kp == 0),
                                    stop=(kp == 1), perf_mode=DR)
                            nc.vector.tensor_scalar(
                                vg8[t // 2][:, :, t % 2, 0:64],
                                pp[:].rearrange("p (h d) -> p h d", h=8),
                                1.0 / WS, None, ALU.mult)
                        for jp in range(4):
                            nc.sync.dma_start(
                                out=vg8[jp][:, :, :, 64:65],
                                in_=ones16[:].rearrange(
                                    "p (h t o) -> p h t o", h=8, t=2))
                        # remap q/k into the [32, 2, V] k-split layout
                        for h in range(8):
                            for sl in range(2):
                                po = (h % 2) * 64 + sl * 32
                                nc.sync.dma_start(
                                    out=q8[h][:, sl, :],
                                    in_=qf8[h // 2][po:po + 32, :])
                                nc.sync.dma_start(
                                    out=k8[h][:, sl, :],
                                    in_=qf8[4 + h // 2][po:po + 32, :])

                # attention: S^T[j,i]; bias accumulated into PSUM by PE;
                # softmax over j (partitions) via ones-row denominator
                with tc.tile_pool(name="attt", bufs=1) as attt, \
                        tc.tile_pool(name="attps", bufs=2,
                                     space="PSUM") as attps, \
                        tc.tile_pool(name="attpo", bufs=1,
                                     space="PSUM") as attpo:
                    for hg in range(4):
                        ops = [attpo.tile([128, V], f32, tag=f"ops{i}",
                                          bufs=1, name=f"ops{i}")
                               for i in range(2)]

                        def emit_pv(st):
                            jp_, h_, Pp_ = st
                            pr = Pp_[:].rearrange("p (two n) -> p two n",
                                                  two=2)
                            for nh in range(2):
                                s = slice(nh * 512, nh * 512 + 512)
                                nc.tensor.matmul(
                                    ops[h_ % 2][:, s],
                                    vg8[jp_][:, h_, :, :],
                                    pr[:, :, s], start=(jp_ == 0),
                                    stop=(jp_ == 3), perf_mode=DR)

                        pend = []
                        Pp_cur = [None, None]
                        for jt in range(8):
                            jsl = slice(jt * 128, jt * 128 + 128)
                            for hi in range(2):
                                h = hg * 2 + hi
                                S = attps.tile([128, V], f32, tag="mms",
                                               name="S")
                                w01 = wid_t[:, h * 512:h * 512 + 256] \
                                    .rearrange("p (two m) -> p two m", two=2)
                                w2z = wid_t[:, h * 512 + 256:h * 512 + 512] \
                                    .rearrange("p (two m) -> p two m", two=2)
                                m01 = mk_t[jt][:, 0:2048] \
                                    .rearrange("p (two n) -> p two n", two=2)
                                m2z = mk_t[jt][:, 2048:4096] \
                                    .rearrange("p (two n) -> p two n", two=2)
                                for nh in range(2):
                                    s = slice(nh * 512, nh * 512 + 512)
                                    nc.tensor.matmul(
                                        S[:, s], k8[h][:, :, jsl],
                                        q8[h][:, :, s],
                                        start=True, stop=False,
                                        perf_mode=DR)
                                for nh in range(2):
                                    s = slice(nh * 512, nh * 512 + 512)
                                    nc.tensor.matmul(
                                        S[:, s], w01, m01[:, :, s],
                                        start=False, stop=False,
                                        perf_mode=DR,
                                        skip_group_check=True)
                                for nh in range(2):
                                    s = slice(nh * 512, nh * 512 + 512)
                                    nc.tensor.matmul(
                                        S[:, s], w2z, m2z[:, :, s],
                                        start=False, stop=True,
                                        perf_mode=DR,
                                        skip_group_check=True)
                                # queued PVs go to the PE here so PE
                                # streams while Act exps the current pair
                                for st in pend:
                                    emit_pv(st)
                                pend = []
                                if jt % 2 == 0:
                                    Pp_cur[hi] = attt.tile(
                                        [128, 2048], f8e5, tag=f"probs{hi}",
                                        bufs=2, name=f"Pp{hi}")
                                Pp = Pp_cur[hi]
                                nc.scalar.activation(
                                    Pp[:, (jt % 2) * 1024:
                                       (jt % 2) * 1024 + 1024],
                                    S, ACTF.Exp, scale=1.0 / (QS * QS))
                                if jt % 2 == 1:
                                    pend.append((jt // 2, h, Pp))
                        for st in pend:
                            emit_pv(st)
                        # divide by the ones-row sums; att scaled by AS
                        # and written fp8 for the proj matmul
                        for hi in range(2):
                            h = hg * 2 + hi
                            ls = attt.tile([1, V], f32, tag="ls",
                                           bufs=2, name="ls")
                            nc.scalar.copy(ls, ops[hi][64:65, :])
                            lT = attt.tile([128, 8], f32, tag="lT",
                                           bufs=2, name="lT")
                            for c in range(8):
                                nc.sync.dma_start(
                                    out=lT[:, c:c + 1],
                                    in_=ls[0:1,
                                           c * 128:(c + 1) * 128])
                            rlT = attt.tile([128, 8], f32, tag="rlT",
                                            bufs=2, name="rlT")
                            nc.vector.reciprocal(rlT, lT)
                            rl_s = attt.tile([1, V], f32, tag="rls",
                                             bufs=2, name="rl_s")
                            for c in range(8):
                                nc.sync.dma_start(
                                    out=rl_s[0:1, c * 128:(c + 1) * 128],
                                    in_=rlT[:, c:c + 1])
                            rlb = attt.tile([64, V], f32, tag="rlb",
                                            bufs=2, name="rlb")
                            nc.gpsimd.partition_broadcast(rlb, rl_s)
                            kp, slot = h // 4, (h // 2) % 2
                            po = (h % 2) * 64
                            dst = att8[kp][po:po + 64,
                                           slot * 1024:slot * 1024 + 1024]
                            nc.vector.scalar_tensor_tensor(
                                dst, ops[hi][0:64, :], AS, rlb,
                                ALU.mult, ALU.mult)

                # proj + residual (fp8 DoubleRow; biases are zero in this
                # model's setup, the 1/(WS*AS) unscale is folded into the
                # residual stt)
                with tc.tile_pool(name="projw", bufs=1) as projw, \
                        tc.tile_pool(name="projp", bufs=4,
                                     space="PSUM") as projp:
                    wp_t = [projw.tile([128, 1024], f8, tag=f"wproj{kp}",
                                       name="wp_t") for kp in range(2)]
                    for kp in range(2):
                        nc.sync.dma_start(
                            out=wp_t[kp],
                            in_=wp8[kp * 128:(kp + 1) * 128, :])
                    wp_r = [wp_t[kp][:].rearrange("p (two m) -> p two m",
                                                  two=2) for kp in range(2)]
                    att_r = [att8[kp][:].rearrange("p (two n) -> p two n",
                                                   two=2) for kp in range(2)]
                    for m in range(4):
                        for nh in range(2):
                            s = slice(nh * 512, nh * 512 + 512)
                            pp = projp.tile([128, 512], f32, tag="mmproj")
                            for kp in range(2):
                                nc.tensor.matmul(
                                    pp,
                                    wp_r[kp][:, :, m * 128:(m + 1) * 128],
                                    att_r[kp][:, :, s],
                                    start=(kp == 0), stop=(kp == 1),
                                    perf_mode=DR)
                            nc.vector.scalar_tensor_tensor(
                                x2[m][:, s], pp, 1.0 / (WS * AS),
                                xT_t[m][:, s].bitcast(f32), ALU.mult,
                                ALU.add)

            # ---------- MLP branch (f32r for accuracy) ----------
            with tc.tile_pool(name="mlplife", bufs=1) as mlplife:
                h2 = [mlplife.tile([128, V], f32r, tag=f"h2{kc}",
                                   name=f"h2_{kc}") for kc in range(4)]
                adaln(x2, 1, [h2[kc][:] for kc in range(4)])
                with tc.tile_pool(name="mlpw", bufs=1) as mlpw, \
                        tc.tile_pool(name="mlpt", bufs=1) as mlpt, \
                        tc.tile_pool(name="mlpp", bufs=4,
                                     space="PSUM") as mlpp:
                    wm1_t = [mlpw.tile([128, 2048], f32r, tag=f"wm1{kc}",
                                       name="wm1_t") for kc in range(4)]
                    for kc in range(4):
                        nc.sync.dma_start(
                            out=wm1_t[kc],
                            in_=wm1[kc * 128:(kc + 1) * 128, :])
                    wm2_t = [mlpw.tile([128, 512], f32r, tag=f"wm2{kc}",
                                       name="wm2_t") for kc in range(16)]
                    for kc in range(16):
                        nc.sync.dma_start(
                            out=wm2_t[kc],
                            in_=wm2[kc * 128:(kc + 1) * 128, :])
                    for nh in range(2):
                        s = slice(nh * 512, nh * 512 + 512)
                        g = [mlpt.tile([128, 512], f32r, tag=f"g{m}",
                                       name=f"g{m}") for m in range(16)]
                        for m in range(16):
                            pp = mlpp.tile([128, 512], f32, tag="mmm1")
                            for kc in range(4):
                                mm(pp, wm1_t[kc][:, m * 128:(m + 1) * 128],
                                   h2[kc][:, s], start=(kc == 0),
                                   stop=(kc == 3))
                            nc.scalar.activation(g[m], pp, ACTF.Gelu,
                                                 bias=bm1_t[:, m:m + 1])
                        for m in range(4):
                            pp = mlpp.tile([128, 512], f32, tag="mmm2")
                            for kc in range(16):
                                mm(pp, wm2_t[kc][:, m * 128:(m + 1) * 128],
                                   g[kc], start=(kc == 0), stop=(kc == 15))
                            yt = mlpt.tile([128, 512], f32, tag="yt",
                                           bufs=2, name="yt")
                            nc.vector.scalar_tensor_tensor(
                                yt, pp, bm2_t[:, m:m + 1],
                                x2[m][:, s].bitcast(f32), ALU.add,
                                ALU.add)
                            nc.sync.dma_start(
                                out=yT[m * 128:(m + 1) * 128, s], in_=yt)

    nc.compile()
    return nc


def _pack_pairs(W, s):
    """[K, M] weight -> fp8 DoubleRow pair layout [K//2, 2*M], scaled."""
    f8np = ml_dtypes.float8_e4m3
    K, M = W.shape
    arr = (W * s).reshape(K // 256, 2, 128, M).transpose(0, 2, 1, 3)
    return np.ascontiguousarray(arr.reshape(K // 2, 2 * M).astype(f8np))


def _make_in_maps(inputs):
    f8np = ml_dtypes.float8_e4m3
    x = np.asarray(inputs["x"], dtype=np.float32)
    cond = np.asarray(inputs["cond"], dtype=np.float32)
    ei = np.asarray(inputs["edge_index"])
    w_qkv = np.asarray(inputs["w_qkv"], dtype=np.float32)
    et = np.asarray(inputs["edge_table"], dtype=np.float32)
    # shift-invariance: remove edge type 3
    cb = et - et[3:4, :]  # [4, H]; row 3 == 0

    for bname in ("b_proj",):
        assert np.abs(np.asarray(inputs[bname])).max() == 0.0, \
            f"{bname} must be zero (folded out of the fp8 proj path)"

    scale = 1.0 / np.sqrt(HD)
    wqk = w_qkv[:, :2 * D].copy()
    wqk[:, :D] *= scale
    wv = np.ascontiguousarray(w_qkv[:, 2 * D:])
    wada = np.concatenate([inputs["w_ada1"], inputs["w_ada2"]],
                          axis=1).astype(ml_dtypes.bfloat16)
    bada = np.concatenate([inputs["b_ada1"], inputs["b_ada2"]]).astype(
        np.float32).copy()
    bada[:D] += 1.0          # fold the (1 + scale) into ada1 scale bias
    bada[2 * D:3 * D] += 1.0  # and ada2 scale bias

    # per-head identity blocks for the PE-side bias accumulate
    eye = np.eye(128, dtype=np.float32)
    wid = np.zeros((128, 4096), dtype=np.float32)
    for h in range(H):
        base = h * 512
        for e in range(3):
            # q/k carry a x16 scale each; bias must match S's x256
            wid[:, base + e * 128: base + (e + 1) * 128] = \
                cb[e, h] * 256.0 * eye
    wid = wid.astype(f8np)

    shared = {
        "onesc": np.ones((128, 8), dtype=np.float32),
        "ones16": np.ones((128, 16), dtype=f8np),
        "wid": wid,
        "wqk8": _pack_pairs(wqk, WS),
        "wv8": _pack_pairs(wv, WS),
        "wp8": _pack_pairs(inputs["w_proj"].astype(np.float32), WS),
        "wada": np.ascontiguousarray(wada),
        "bada": np.ascontiguousarray(bada.reshape(16, 128)),
        "wm1": np.ascontiguousarray(inputs["w_mlp1"].astype(np.float32)),
        "bm1": np.ascontiguousarray(
            inputs["b_mlp1"].astype(np.float32).reshape(16, 128)),
        "wm2": np.ascontiguousarray(inputs["w_mlp2"].astype(np.float32)),
        "bm2": np.ascontiguousarray(
            inputs["b_mlp2"].astype(np.float32).reshape(4, 128)),
    }
    in_maps = []
    for b in range(B):
        eiT = ei[b].T  # [j, i]
        mkb = np.zeros((V, 4 * V), dtype=f8np)
        for e in range(3):
            mkb[:, e * V:(e + 1) * V] = (eiT == e).astype(f8np)
        in_maps.append(dict(
            shared,
            xT=np.ascontiguousarray(x[b].T),
            mk=mkb,
            condc=np.ascontiguousarray(cond[b].reshape(4, 128)),
        ))
    return in_maps


def kernel(**inputs):
    from concourse.bass_utils import run_bass_kernel_spmd

    if "prog" not in _cache:
        _cache["prog"] = _build_program()
    nc = _cache["prog"]

    in_maps = _make_in_maps(inputs)
    res = run_bass_kernel_spmd(nc, in_maps, core_ids=list(range(NCORES)))
    out = np.stack([np.ascontiguousarray(res.results[b]["yT"].T)
                    for b in range(B)])
    return out.astype(np.float32)
